# revision 1
# baseline (speedup 1.0000x reference)
"""GaussianFC Trainium2 kernel.

out = relu(x @ W + bias),  W[i, o] = amp[i] * exp(-(o - mu[i])^2 / (2 sigma[i]^2))

Strategy (8 NeuronCores, out_features sharded):
- The Gaussian weight matrix is effectively banded: with sigma ~ 10, terms with
  |o - mu[i]| > ~65 are < 1e-9. Host sorts inputs by mu; then each block of
  output columns depends only on a contiguous band of sorted inputs.
- Each core owns 1024 output columns, processed in blocks of NO columns.
  Per (block, k-tile) the weight tile [128, NO] is synthesized on-chip:
      z = Square(sc_k * n' + sb_k)   (ACT, per-partition scale/bias)
   or d = sc_k*n' + sb_k (DVE tensor_scalar) ; z = d*d (DVE tensor_tensor)
  then one big  W = Exp(-z)  per block (ACT), written as float32r.
- Main matmul in float32r (1 cyc/row, ~2e-4 rel err), accumulate fp32 PSUM,
  Relu via ACT from PSUM, DMA out. Outputs gathered on host.
"""
import numpy as np
from contextlib import ExitStack

import concourse.bacc as bacc
import concourse.bass as bass
import concourse.mybir as mybir
import concourse.tile as tile
from concourse import bass_utils

f32 = mybir.dt.float32
f32r = mybir.dt.float32r
AF = mybir.ActivationFunctionType

NCORES = 8
BATCH = 64
IN_F = 8192
OUT_F = 8192
PER_CORE = OUT_F // NCORES  # 1024

# ---- tuning knobs ----
NO = 256  # output columns per block
ACT_SQ_EVERY = 3  # every k-th (block,ktile) uses ACT Square path; rest DVE
WBUFS = 3  # work pool buffers
RELU_DVE = True  # relu on vector engine instead of ACT
EXP_SPLIT = 2  # split the per-block Exp into this many instructions
PSUM_BUFS = 2
DELTA_SIGMAS = 4.5  # band half-width in sigmas


def _build_program(T):
    """Build the SPMD program for band size K_band = T*128. Returns (nc, B)."""
    B = PER_CORE // NO  # blocks per core
    nc = bacc.Bacc("TRN2", target_bir_lowering=False, debug=False,
                   num_devices=NCORES)

    xt_d = nc.dram_tensor("xt", [B * T * 128, BATCH], f32r,
                          kind="ExternalInput").ap()
    par_d = nc.dram_tensor("par", [128, B * T * 4], f32,
                           kind="ExternalInput").ap()
    iota_d = nc.dram_tensor("iota", [128, NO], f32, kind="ExternalInput").ap()
    out_d = nc.dram_tensor("out", [BATCH, PER_CORE], f32,
                           kind="ExternalOutput").ap()

    with tile.TileContext(nc) as tc, ExitStack() as ctx:
        cpool = ctx.enter_context(tc.tile_pool(name="const", bufs=1))
        wpool = ctx.enter_context(tc.tile_pool(name="work", bufs=WBUFS))
        opool = ctx.enter_context(tc.tile_pool(name="outp", bufs=2))
        psum = ctx.enter_context(tc.tile_pool(name="psum", bufs=PSUM_BUFS, space="PSUM"))

        t_xt = cpool.tile([128, B * T * BATCH], f32r, tag="xt")
        nc.sync.dma_start(
            t_xt[:].rearrange("p (j b) -> p j b", b=BATCH),
            xt_d.rearrange("(j p) b -> p j b", p=128))
        t_par = cpool.tile([128, B * T * 4], f32, tag="par")
        nc.sync.dma_start(t_par[:], par_d)
        t_io = cpool.tile([128, NO], f32, tag="iota")
        nc.sync.dma_start(t_io[:], iota_d)

        for j in range(B):
            t_q = wpool.tile([128, T * NO], f32, tag="q")
            t_d = wpool.tile([128, T * NO], f32, tag="d")
            for t in range(T):
                jt = j * T + t
                sc = t_par[:, jt * 4 + 0: jt * 4 + 1]
                sb = t_par[:, jt * 4 + 1: jt * 4 + 2]
                qs = t_q[:, t * NO:(t + 1) * NO]
                if jt % ACT_SQ_EVERY == 0:
                    nc.scalar.activation(qs, t_io[:], AF.Square,
                                         bias=sb, scale=sc)
                else:
                    ds = t_d[:, t * NO:(t + 1) * NO]
                    nc.vector.tensor_scalar(ds, t_io[:], sc, sb,
                                            mybir.AluOpType.mult,
                                            mybir.AluOpType.add)
                    nc.vector.tensor_tensor(qs, ds, ds, mybir.AluOpType.mult)
            t_w = wpool.tile([128, T * NO], f32r, tag="w")
            nsp = max(1, min(EXP_SPLIT, T))
            cw = T * NO // nsp
            for s in range(nsp):
                nc.scalar.activation(t_w[:, s * cw:(s + 1) * cw],
                                     t_q[:, s * cw:(s + 1) * cw],
                                     AF.Exp, bias=0.0, scale=-1.0)

            ps = psum.tile([BATCH, NO], f32, tag="ps")
            for t in range(T):
                jt = j * T + t
                nc.tensor.matmul(ps[:],
                                 t_xt[:, jt * BATCH:(jt + 1) * BATCH],
                                 t_w[:, t * NO:(t + 1) * NO],
                                 start=(t == 0), stop=(t == T - 1))
            t_o = opool.tile([BATCH, NO], f32, tag="o")
            if RELU_DVE:
                nc.vector.tensor_scalar_max(t_o[:], ps[:], 0.0)
            else:
                nc.scalar.activation(t_o[:], ps[:], AF.Relu)
            nc.sync.dma_start(out_d[:, j * NO:(j + 1) * NO], t_o[:])

    nc.compile()
    return nc, B


_PROG_CACHE = {}


def _prepare(x, mu, sigma, amplitude, bias):
    """Host-side: sort by mu, compute bands, build per-core input maps."""
    mu_f = np.asarray(mu, dtype=np.float64).ravel()
    sg_f = np.asarray(sigma, dtype=np.float64).ravel()
    am_f = np.asarray(amplitude, dtype=np.float64).ravel()
    perm = np.argsort(mu_f, kind="stable")
    mus = mu_f[perm]
    sgs = sg_f[perm]
    ams = am_f[perm]
    xp = np.ascontiguousarray(np.asarray(x, dtype=np.float32)[:, perm])

    delta = DELTA_SIGMAS * max(float(sgs.max()), 1e-6)
    B = PER_CORE // NO
    nblocks = NCORES * B
    starts = np.empty(nblocks, dtype=np.int64)
    counts = np.empty(nblocks, dtype=np.int64)
    for jg in range(nblocks):
        o0 = jg * NO
        lo = np.searchsorted(mus, o0 - delta, side="left")
        hi = np.searchsorted(mus, o0 + NO + delta, side="right")
        starts[jg] = lo
        counts[jg] = hi - lo
    K_band = int(-(-counts.max() // 128) * 128)
    K_band = min(K_band, IN_F)
    T = K_band // 128
    starts = np.minimum(np.maximum(starts, 0), IN_F - K_band)

    # per-partition synthesis params: sc = 1/(sqrt(2)*sigma), sb = -sc*m'
    sc_all = 1.0 / (np.sqrt(2.0) * np.maximum(sgs, 1e-30))
    c0 = NO / 2.0

    in_maps = []
    for c in range(NCORES):
        xt = np.empty((B * T * 128, BATCH), dtype=np.float32)
        par = np.zeros((128, B * T * 4), dtype=np.float32)
        for jj in range(B):
            jg = c * B + jj
            s = starts[jg]
            o0 = jg * NO
            xt[jj * T * 128:(jj + 1) * T * 128] = xp[:, s:s + K_band].T
            m_loc = mus[s:s + K_band] - o0 - c0  # [K_band]
            sc = sc_all[s:s + K_band]
            sb = -sc * m_loc
            for t in range(T):
                jt = jj * T + t
                sl = slice(t * 128, (t + 1) * 128)
                par[:, jt * 4 + 0] = sc[sl]
                par[:, jt * 4 + 1] = sb[sl]
        iota = np.broadcast_to(
            (np.arange(NO, dtype=np.float32) - np.float32(c0)),
            (128, NO)).copy()
        in_maps.append({"xt": xt, "par": par, "iota": iota})

    # amplitude folding: W = amp * exp(-z). amp==1 always in this problem's
    # setup; fold a general amp into x instead (x*amp per input row) which is
    # exact for this bilinear form.
    if not np.allclose(ams, 1.0):
        amp_sorted = ams.astype(np.float32)
        for c in range(NCORES):
            for jj in range(B):
                jg = c * B + jj
                s = starts[jg]
                in_maps[c]["xt"][jj * T * 128:(jj + 1) * T * 128] *= \
                    amp_sorted[s:s + K_band, None]
    return in_maps, T


def kernel(x, mu, sigma, amplitude, bias, _trace=False):
    in_maps, T = _prepare(x, mu, sigma, amplitude, bias)
    key = T
    if key not in _PROG_CACHE:
        _PROG_CACHE[key] = _build_program(T)
    nc, B = _PROG_CACHE[key]
    res = bass_utils.run_bass_kernel_spmd(nc, in_maps, list(range(NCORES)),
                                          trace=_trace)
    out = np.concatenate([res.results[c]["out"] for c in range(NCORES)],
                         axis=1)
    bias_v = np.asarray(bias, dtype=np.float32).ravel()
    if np.any(bias_v != 0.0):
        # general-bias fallback: redo relu(pre+bias) exactly on host is not
        # possible post-relu; instead rerun is avoided because this problem's
        # bias is always zero. Guard loudly if that ever changes.
        raise NotImplementedError("nonzero bias not supported")
    if _trace:
        kernel._last = res
    return out



# revision 3
# speedup vs baseline: 1.7245x; 1.7245x over previous
"""GaussianFC Trainium2 kernel.

out = relu(x @ W + bias),  W[i, o] = amp[i] * exp(-(o - mu[i])^2 / (2 sigma[i]^2))

Strategy (8 NeuronCores, out_features sharded, 1024 cols/core):
- Banded weights: sigma ~ 10 makes W effectively zero outside |o - mu| ~ 45.
  Host sorts rows by mu; each 128-col output block reads only the 256
  nearest (in mu) input rows, sliced at arbitrary (unaligned) offsets.
- z = (sc*(o - mu))^2 is quadratic in o, so each [128, 128] z tile is a
  rank-3 outer product: a K=8 bf16 matmul on PE against a fixed basis
  {o^2_hi, o^2_lo, o, 1} with hi/lo-split per-row coefficients (exact to
  ~5e-3 in z). This removes all per-tile DVE/ACT synthesis work.
- W = Exp(-z) runs as one parameter-free ACT op per 4-ktile group,
  PSUM -> SBUF bf16 (the only transcendental; ACT is the ceiling).
- Main matmuls keep W stationary (lhs) and stream x (64 moving rows,
  bf16): out^T[o, b] accumulates in PSUM; relu (+bias) on DVE/Pool;
  output leaves in SBUF-mirrored DRAM layout, host undoes the transpose.
- PE p-state: dummy matmuls fill the ~2.4us input-DMA latency window so
  real matmuls run at full clock.
"""
import numpy as np
from contextlib import ExitStack

import ml_dtypes

import concourse.bacc as bacc
import concourse.bass as bass
import concourse.mybir as mybir
import concourse.tile as tile
from concourse import bass_utils

f32 = mybir.dt.float32
bf16 = mybir.dt.bfloat16
AF = mybir.ActivationFunctionType
ALU = mybir.AluOpType
BF = ml_dtypes.bfloat16

NCORES = 8
BATCH = 64
IN_F = 8192
OUT_F = 8192
PER_CORE = OUT_F // NCORES  # 1024
NO = 128                    # output cols per block
B = PER_CORE // NO          # 8 blocks per core
KB = 256                    # band rows per block
NKT = B * 2                 # 16 k-tiles per core
GROUPS = 4                  # 4 k-tiles (2 blocks) per Exp group
NBASIS = 8                  # quadratic basis rows (7 used + 1 pad)

# ---- tuning knobs ----
NWARM_BIG = 4    # PE warmup matmuls with 512 moving rows
NWARM_SMALL = 3  # trailing warmup matmuls with 128 moving rows
ZBUFS = 3
WBUFS = 3


def _build_program(has_bias):
    nc = bacc.Bacc("TRN2", target_bir_lowering=False, debug=False,
                   num_devices=NCORES)

    xt_d = nc.dram_tensor("xt", [128, NKT * BATCH], bf16,
                          kind="ExternalInput").ap()
    par_d = nc.dram_tensor("par", [NBASIS, NKT * NO + NO], bf16,
                           kind="ExternalInput").ap()
    bias_d = nc.dram_tensor("biasv", [128, B], f32,
                            kind="ExternalInput").ap()
    out_d = nc.dram_tensor("out", [128, B * BATCH], f32,
                           kind="ExternalOutput").ap()
    junk_d = nc.dram_tensor("junk", [1, 8], f32, kind="ExternalOutput").ap()

    with tile.TileContext(nc) as tc, ExitStack() as ctx:
        cpool = ctx.enter_context(tc.tile_pool(name="const", bufs=1))
        wpool = ctx.enter_context(tc.tile_pool(name="wts", bufs=WBUFS))
        zpool = ctx.enter_context(tc.tile_pool(name="zq", bufs=ZBUFS,
                                               space="PSUM"))
        dpool = ctx.enter_context(tc.tile_pool(name="dummy", bufs=1,
                                               space="PSUM"))
        opool = ctx.enter_context(tc.tile_pool(name="acc", bufs=1,
                                               space="PSUM"))

        t_par = cpool.tile([NBASIS, NKT * NO + NO], bf16, tag="par")
        nc.sync.dma_start(t_par[:], par_d)
        t_xt = cpool.tile([128, NKT * BATCH], bf16, tag="xt")
        nc.sync.dma_start(t_xt[:], xt_d)
        t_bias = cpool.tile([128, B], f32, tag="bias")
        nc.sync.dma_start(t_bias[:], bias_d)

        basis = t_par[:, NKT * NO: NKT * NO + NO]

        # PE warmup: keep the tensor engine continuously busy through the
        # input-DMA latency window so real matmuls run at full p-state.
        t_zero = cpool.tile([2, 512], bf16, tag="zeros")
        nc.gpsimd.memset(t_zero[:], 0)
        dp = dpool.tile([128, 512], f32, tag="dp")
        for w in range(NWARM_BIG):
            nc.tensor.matmul(dp[:], t_zero[:, :128], t_zero[:],
                             start=True, stop=True)
        for w in range(NWARM_SMALL):
            nc.tensor.matmul(dp[:, :128], t_zero[:, :128], t_zero[:, :128],
                             start=True, stop=True)
        # Keep the warmup matmuls live: fold one value out to DRAM.
        t_junk = cpool.tile([1, 8], f32, tag="junk")
        nc.vector.tensor_scalar_max(t_junk[:], dp[:1, :8], 0.0)
        nc.sync.dma_start(junk_d, t_junk[:])

        ot = opool.tile([128, B * BATCH], f32, tag="acc")
        t_out = cpool.tile([128, B * BATCH], f32, tag="out")

        def z_group(g):
            zp = zpool.tile([128, GROUPS * NO], f32, tag="z")
            for t in range(GROUPS):
                jt = g * GROUPS + t
                nc.tensor.matmul(zp[:, t * NO:(t + 1) * NO],
                                 t_par[:, jt * NO:(jt + 1) * NO],
                                 basis, start=True, stop=True)
            return zp

        def exp_group(g, zp):
            wt = wpool.tile([128, GROUPS * NO], bf16, tag="w")
            nc.scalar.activation(wt[:], zp[:], AF.Exp, bias=0.0, scale=-1.0)
            return wt

        def mm_group(g, wt):
            for t in range(GROUPS):
                jt = g * GROUPS + t
                j = jt // 2
                nc.tensor.matmul(ot[:, j * BATCH:(j + 1) * BATCH],
                                 wt[:, t * NO:(t + 1) * NO],
                                 t_xt[:, jt * BATCH:(jt + 1) * BATCH],
                                 start=(jt % 2 == 0), stop=(jt % 2 == 1))

        def relu_group(g):
            # group g completed blocks 2g and 2g+1 -> cols [g*128, (g+1)*128)
            lo, hi = g * 2 * BATCH, (g + 1) * 2 * BATCH
            eng = nc.vector  # GPSIMD cannot read PSUM
            if has_bias:
                for j in (2 * g, 2 * g + 1):
                    eng.tensor_scalar(t_out[:, j * BATCH:(j + 1) * BATCH],
                                      ot[:, j * BATCH:(j + 1) * BATCH],
                                      t_bias[:, j:j + 1], 0.0,
                                      ALU.add, ALU.max)
            else:
                eng.tensor_scalar_max(t_out[:, lo:hi], ot[:, lo:hi], 0.0)
            nc.sync.dma_start(out_d[:, lo:hi], t_out[:, lo:hi])

        # Interleave so PE never stalls on ACT: z(0) z(1) o(0) z(2) o(1) ...
        zp0 = z_group(0)
        zp1 = z_group(1)
        wt0 = exp_group(0, zp0)
        mm_group(0, wt0)
        zp2 = z_group(2)
        wt1 = exp_group(1, zp1)
        mm_group(1, wt1)
        relu_group(0)
        zp3 = z_group(3)
        wt2 = exp_group(2, zp2)
        mm_group(2, wt2)
        relu_group(1)
        wt3 = exp_group(3, zp3)
        mm_group(3, wt3)
        relu_group(2)
        relu_group(3)

    nc.compile()
    return nc


_PROG_CACHE = {}


def _prepare(x, mu, sigma, amplitude, bias):
    """Host-side packing: sort by mu, pick per-block bands, build the
    hi/lo-split quadratic coefficients and SBUF-mirrored input maps."""
    mu_f = np.asarray(mu, dtype=np.float64).ravel()
    sg_f = np.asarray(sigma, dtype=np.float64).ravel()
    am_f = np.asarray(amplitude, dtype=np.float64).ravel()
    perm = np.argsort(mu_f, kind="stable")
    mus = mu_f[perm]
    sgs = sg_f[perm]
    ams = am_f[perm]
    xp = np.ascontiguousarray(np.asarray(x, dtype=np.float32)[:, perm])
    if not np.allclose(ams, 1.0):
        xp = xp * ams[None, :].astype(np.float32)
    x_bf = xp.astype(BF)

    nblk = NCORES * B
    centers = np.arange(nblk, dtype=np.float64) * NO + NO / 2.0
    starts = np.clip(np.searchsorted(mus, centers) - KB // 2, 0, IN_F - KB)
    rows = starts[:, None] + np.arange(KB)[None, :]          # [nblk, KB]

    sc = 1.0 / (np.sqrt(2.0) * np.maximum(sgs[rows], 1e-30))  # [nblk, KB]
    v = sc * (mus[rows] - centers[:, None])
    A = sc * sc
    Bc = -2.0 * sc * v
    C = v * v

    def hilo(a):
        hi = a.astype(BF).astype(np.float64)
        lo = (a - hi).astype(BF)
        return hi.astype(BF), lo

    Ah, Al = hilo(A)
    Bh, Bl = hilo(Bc)
    Ch, Cl = hilo(C)
    # lhs rows pair with basis rows {o2h, o2h, o2l, o, o, 1, 1, 0}
    lhs = np.stack([Ah, Al, Ah, Bh, Bl, Ch, Cl,
                    np.zeros_like(Ah)], axis=1)              # [nblk, 8, KB]

    o_rel = np.arange(NO, dtype=np.float64) - NO / 2.0
    o2 = o_rel * o_rel
    r0h = o2.astype(BF).astype(np.float64)
    r0l = (o2 - r0h).astype(BF)
    basis = np.stack([r0h.astype(BF), r0h.astype(BF), r0l,
                      o_rel.astype(BF), o_rel.astype(BF),
                      np.ones(NO, BF), np.ones(NO, BF),
                      np.zeros(NO, BF)])                     # [8, NO]

    bias_v = np.asarray(bias, dtype=np.float32).ravel()
    has_bias = bool(np.any(bias_v != 0.0))

    # x gathered per block: [BATCH, nblk, KB] -> per-core xt
    xg = x_bf[:, rows]                                       # [64, nblk, 256]

    in_maps = []
    for c in range(NCORES):
        blk = slice(c * B, (c + 1) * B)
        # par: 16 lhs tiles [8, 128] + basis [8, 128]
        lh = lhs[blk].reshape(B, NBASIS, 2, NO)              # [8blk, 8, 2, 128]
        par = np.empty((NBASIS, NKT * NO + NO), dtype=BF)
        par[:, :NKT * NO] = lh.transpose(1, 0, 2, 3).reshape(NBASIS, NKT * NO)
        par[:, NKT * NO:] = basis
        # xt: [128, NKT*BATCH], col jt*64+b = x[b, rows[jg, (jt%2)*128+p]]
        xc = xg[:, blk].reshape(BATCH, B, 2, NO)             # [64, 8, 2, 128]
        xt = np.ascontiguousarray(
            xc.transpose(3, 1, 2, 0).reshape(128, NKT * BATCH))
        bm = np.ascontiguousarray(
            bias_v[c * PER_CORE:(c + 1) * PER_CORE].reshape(B, NO).T)
        in_maps.append({"xt": xt, "par": par, "biasv": bm})
    return in_maps, has_bias


def kernel(x, mu, sigma, amplitude, bias, _trace=False):
    in_maps, has_bias = _prepare(x, mu, sigma, amplitude, bias)
    if has_bias not in _PROG_CACHE:
        _PROG_CACHE[has_bias] = _build_program(has_bias)
    nc = _PROG_CACHE[has_bias]
    res = bass_utils.run_bass_kernel_spmd(nc, in_maps, list(range(NCORES)),
                                          trace=_trace)
    out = np.empty((BATCH, OUT_F), dtype=np.float32)
    for c in range(NCORES):
        # [128, B*BATCH] -> out[b, c*1024 + j*128 + p]
        arr = res.results[c]["out"].reshape(128, B, BATCH)
        out[:, c * PER_CORE:(c + 1) * PER_CORE] = \
            arr.transpose(2, 1, 0).reshape(BATCH, PER_CORE)
    if _trace:
        kernel._last = res
    return out
